# revision 33
# baseline (speedup 1.0000x reference)
"""Trainium2 Bass kernel for C = tril(tril(A) @ tril(B)), N=4096, fp32.

Sharding: row-parallel x 2-way k-split over 8 cores. Cores 0-3 handle
even k-blocks, cores 4-7 odd k-blocks (host sums the two partial C's).
Parity is pure data: global k-block = 2t + parity for local index t,
so one SPMD program serves both groups, fed parity-packed inputs.

Each core has 8 slots; slot r of core group member c' owns block-row
4r + c' (ibar_r = 4r+3, so slot r sweeps bands J0 = 0..r, local t in
[2*J0, 2r+1]). Smaller rows in a slot band harmlessly compute exact
zeros because A/B are pre-masked (tril) on the host.

Precision: pure fp16 inputs, fp32 PSUM accumulate, fp16 partial-C
output (host upcasts and sums the two parities). Measured rel err
~5e-4 vs the 2e-2 gate.

DMA plan (all HWDGE; only the sync+scalar rings exist on TRN2, each
sustaining ~150-165 GB/s, and both are ~100% busy for the whole body):
 - B is packed on the host into per-band tiles with big per-partition
   lines for ring efficiency: a trimmed "head" tile per band (the
   first 2 or 4 local t's, with the tril-trimmed 256-wide first step),
   pair tiles for band 0's catch-up window, and [128 x 4 x 512]
   t-quad tiles (4KB lines) for the rest.
 - The startup window is supply-bound: both rings must jointly
   deliver A's head and band 0/1's B in consumption order, so the
   loads are explicitly hand-scheduled per ring (sync_plan /
   scalar_plan) with A tiles split per t-half across the rings. The
   PE warmup burst is sized to end right when the first tiles land.
 - C partials staged fp32->fp16 by vector (DVE cast), DMA'd on the
   alternating rings; loads are issued before all outputs on each
   ring so outputs never block a load.
Bands run in ORDER ending with band 7 (single output, cast split in
two with the DMA split across both rings) so the post-matmul tail is
minimal; REN remaps each band's PSUM tags onto banks guaranteed idle
at that point.
"""
import contextlib
import numpy as np

import concourse.mybir as mybir
import concourse.tile as tile
from concourse import bacc
from concourse.bass_utils import run_bass_kernel_spmd

NB = 4096          # matrix size
P = 128            # partition / block size
KB = NB // P       # 32 global k-blocks
TL = KB // 2       # 16 local k-indices per parity
NCORES = 8
NSLOT = 8          # row-block slots per core (half-rows)
JT = 512           # band width / matmul free-dim tile

NWARM = 6          # PE clock-ramp warmup matmuls (512 free dim each)

# Band order: big bands early, tiny band 7 last (1 output tile) so the
# end-of-kernel cast+DMA tail is minimal; small bands interleaved so
# output DMAs spread. REN keeps adjacent bands' PSUM banks disjoint
# or long-drained.
ORDER = (0, 1, 2, 5, 3, 6, 4, 7)
REN = {0: 0, 1: 0, 2: 0, 5: 5, 3: 0, 6: 6, 4: 0, 7: 7}

LAST_RESULTS = None  # test harness reads exec_time_ns from here
PROFILE_CM = None    # optional: test harness sets a contextmanager factory

_NC_CACHE = {}

F16 = mybir.dt.float16
F32 = mybir.dt.float32


def _band_tiles(J0):
    """B tile schedule for band J0: ('head', t0, n_t, elems) then
    ('quad', t0) entries, in consumption order. Head covers the first
    2 (odd J0) or 4 (even J0) t's with the first step trimmed to 256
    cols; quads cover 4 t's at full 512 width (4KB lines)."""
    t0 = 2 * J0
    tiles = []
    nh = 4 if J0 % 2 == 0 else 2
    nh = min(nh, TL - t0)
    tiles.append(("head", t0, nh, 256 + (nh - 1) * JT))
    t = t0 + nh
    if J0 == 0:
        # band 0's t=4..7 are consumed while the rings are still
        # catching up -- deliver them at pair granularity
        tiles.append(("pair", t, 2, 2 * JT))
        tiles.append(("pair", t + 2, 2, 2 * JT))
        t += 4
    while t < TL:
        tiles.append(("quad", t, 4, 4 * JT))
        t += 4
    return tiles


def _build():
    nc = bacc.Bacc("TRN2")
    M = NSLOT * P  # 1024 packed A columns

    # Parity-packed inputs: local k index t on the leading axis; B is
    # packed as per-band head/quad tiles in issue order (see kernel()).
    ATh_d = nc.dram_tensor("ATh", [TL // 2, P, 2, M], F16, kind="ExternalInput")
    nquad = sum(1 for J0 in range(NSLOT) for k in _band_tiles(J0) if k[0] == "quad")
    npair = sum(1 for J0 in range(NSLOT) for k in _band_tiles(J0) if k[0] == "pair")
    Bq_d = nc.dram_tensor("Bq", [nquad, P, 4 * JT], F16, kind="ExternalInput")
    Bp_d = nc.dram_tensor("Bp", [npair, P, 2 * JT], F16, kind="ExternalInput")
    Bhe_d = nc.dram_tensor("Bhe", [4, P, 256 + 3 * JT], F16, kind="ExternalInput")
    Bho_d = nc.dram_tensor("Bho", [4, P, 256 + JT], F16, kind="ExternalInput")
    C_d = nc.dram_tensor("C", [M, NB], F16, kind="ExternalOutput")

    with tile.TileContext(nc) as tc:
        with (
            tc.tile_pool(name="ares", bufs=1) as ares,
            tc.tile_pool(name="bres", bufs=1) as bres,
            tc.tile_pool(name="obuf", bufs=24) as obuf,
            tc.tile_pool(name="psum", bufs=1, space="PSUM") as psum,
        ):
            # PE warmup: a short burst of throwaway matmuls bridges
            # the idle preamble so the HAM clock governor reaches full
            # rate by the time real data lands (a cold PE runs the
            # first ~3us of matmuls at reduced clock otherwise). warm
            # reads raw (uninitialized) SBUF: no memset dependency, so
            # the burst starts the moment the PE preamble ends; the
            # result (garbage) goes to a throwaway PSUM tile.
            warm = nc.alloc_sbuf_tensor([P, JT], F16)
            ps_w = psum.tile([P, JT], F32, tag="ps0", name="ps_warm")
            for _ in range(NWARM):
                nc.tensor.matmul(ps_w[:], warm.ap()[:, :P], warm.ap()[:],
                                 start=True, stop=True)
            # zero tile for mid-stream keep-warm pads: a matmul with an
            # all-zero stationary adds exactly 0 to a live accumulator,
            # so it can fill predictable early supply-stall slots and
            # keep the PE clock governor ramped without changing
            # results. (Must be a real memset -- NaN garbage would
            # poison accumulators via 0*NaN.)
            zpad = ares.tile([P, JT], F16, tag="zpad", name="zpad")
            nc.gpsimd.memset(zpad[:], 0.0)

            # Input loads: A resident t-pair tiles + B per-band
            # head/pair/quad tiles. Tiles are declared here; the DMAs
            # are emitted below via the per-ring plans.
            ah = [ares.tile([P, 2, M], F16, tag=f"ah{tp}", name=f"ah{tp}")
                  for tp in range(TL // 2)]
            bt = {}
            bload = {}
            iq = ip = ihe = iho = 0
            border = []
            for J0 in ORDER:
                for kind, t0, n_t, elems in _band_tiles(J0):
                    b = bres.tile([P, elems], F16, tag=f"b{J0}_{t0}",
                                  name=f"b{J0}_{t0}")
                    if kind == "head":
                        src = (Bhe_d[ihe] if n_t > 2 else Bho_d[iho])[:, :elems]
                        if n_t > 2:
                            ihe += 1
                        else:
                            iho += 1
                        off = 0
                        for i in range(n_t):
                            w = 256 if i == 0 else JT
                            bt[(J0, t0 + i)] = (b, off, w)
                            off += w
                    else:
                        if kind == "quad":
                            src = Bq_d[iq, :, :]
                            iq += 1
                        else:
                            src = Bp_d[ip, :, :]
                            ip += 1
                        for i in range(n_t):
                            bt[(J0, t0 + i)] = (b, i * JT, JT)
                    bload[(J0, t0)] = (b, src, kind, n_t, elems)
                    border.append((J0, t0))

            def emit_b(key, per_t):
                b, src, kind, n_t, elems = bload[key]
                jobs = []
                if per_t and kind == "head":
                    off = 0
                    for i in range(n_t):
                        w = 256 if i == 0 else JT
                        jobs.append((w * P * 2,
                                     lambda eng, b=b, src=src, o=off, w=w:
                                     eng.dma_start(b[:, o:o + w],
                                                   src[:, o:o + w])))
                        off += w
                else:
                    jobs.append((elems * P * 2,
                                 lambda eng, b=b, src=src:
                                 eng.dma_start(b[:], src)))
                return jobs

            def a_half(tp, i):
                a = ah[tp]
                return lambda eng: eng.dma_start(
                    a[:, i, tp * P:], ATh_d[tp, :, i, tp * P:])

            def a_full(tp):
                a = ah[tp]
                return lambda eng: eng.dma_start(
                    a[:, :, tp * P:], ATh_d[tp, :, :, tp * P:])

            def a0_chunk(o0, o1):
                return lambda eng: eng.dma_start(
                    ah[0][:, 0, o0:o1], ATh_d[0, :, 0, o0:o1])

            # Explicit per-ring issue plans, hand-scheduled so every
            # tile's completion (issue ~0.65us + ring transfer at
            # ~165GB/s + ~0.3us receipt) beats its first consumption in
            # the matmul stream. Early chunks are medium-sized: smaller
            # starves the issue pipeline, bigger delays the first uses.
            h = M // 2
            b0 = emit_b((0, 0), True)           # t=0..3 singles
            sync_plan = (
                [b0[0][1], a0_chunk(h, M), b0[1][1], a_half(1, 0),
                 b0[2][1], b0[3][1],
                 emit_b((0, 4), False)[0][1],
                 emit_b((0, 6), False)[0][1],
                 emit_b((0, 8), False)[0][1],
                 emit_b((0, 12), False)[0][1]]
            )
            scalar_plan = (
                [a0_chunk(0, h), a_half(0, 1), a_half(1, 1),
                 a_half(2, 0), a_half(2, 1),
                 a_half(3, 0), a_half(3, 1), a_full(4), a_full(5)]
            )
            # remaining bands in ORDER, alternating rings; late A tiles
            # slotted onto scalar after band 1's B.
            done = {(0, t) for t in (0, 4, 6, 8, 12)}
            rest = [k for k in border if k not in done]
            late_a = [a_full(tp) for tp in (6, 7)]
            flip = 0
            for key in rest:
                fn = emit_b(key, False)[0][1]
                (scalar_plan if flip else sync_plan).append(fn)
                flip ^= 1
                if late_a:
                    scalar_plan.append(late_a.pop(0))
            for fn in sync_plan:
                fn(nc.sync)
            for fn in scalar_plan:
                fn(nc.scalar)

            # keep-warm pad schedule: (band, t) -> number of 512-wide
            # zero matmuls emitted after that t-group. Sized to ~60% of
            # the median PE idle observed at each early supply stall
            # (the rings deliver only ~0.5MB by ~12us; t2..t8 data
            # trails the PE through band 0).
            PADS = {(0, 1): 3, (0, 2): 1, (0, 3): 2, (0, 5): 2,
                    (0, 7): 1, (1, 3): 1}

            nout = 0
            for J0 in ORDER:
                live = range(J0, NSLOT)
                ren = REN[J0]
                ps = {r: psum.tile([P, JT], F32, tag=f"ps{r - ren}",
                                   name=f"ps{r}_{J0}")
                      for r in live}
                for t in range(2 * J0, TL):
                    first = t == 2 * J0
                    if not first:
                        # pads accumulate +0 into slot 7's live chain
                        # (slot 7 is live in every band and only stops
                        # at t=15, so mid-chain insertion is safe)
                        for _ in range(PADS.get((J0, t - 1), 0)):
                            nc.tensor.matmul(ps[NSLOT - 1][:],
                                             zpad[:, :P], zpad[:],
                                             start=False, stop=False)
                    b, off, w = bt[(J0, t)]
                    rhs = b[:, off:off + w]
                    for r in live:
                        if 2 * r + 1 < t:
                            continue
                        last = t == 2 * r + 1
                        nc.tensor.matmul(ps[r][:, :w],
                                         ah[t // 2][:, t % 2,
                                                    r * P:(r + 1) * P],
                                         rhs, start=first, stop=last)
                        if last:
                            ot = obuf.tile([P, JT], F16, tag="o",
                                           name=f"o{r}_{J0}")
                            crows = C_d[r * P:(r + 1) * P,
                                        J0 * JT:(J0 + 1) * JT]
                            if J0 == ORDER[-1]:
                                # final output: two half casts (the
                                # first DMA issues while the second
                                # half casts) with the DMA split
                                # across both rings to minimize the
                                # post-matmul tail
                                hw_ = JT // 2
                                nc.vector.tensor_copy(ot[:, :hw_],
                                                      ps[r][:, :hw_])
                                nc.sync.dma_start(crows[:, :hw_],
                                                  ot[:, :hw_])
                                nc.vector.tensor_copy(ot[:, hw_:],
                                                      ps[r][:, hw_:])
                                nc.scalar.dma_start(crows[:, hw_:],
                                                    ot[:, hw_:])
                            else:
                                nc.vector.tensor_copy(ot[:], ps[r][:])
                                oeng = nc.sync if nout % 2 else nc.scalar
                                nout += 1
                                oeng.dma_start(crows, ot[:])
    nc.finalize()
    return nc


def kernel(A, B):
    global LAST_RESULTS
    A = np.asarray(A, dtype=np.float32)
    B = np.asarray(B, dtype=np.float32)

    if "nc" not in _NC_CACHE:
        _NC_CACHE["nc"] = _build()
    nc = _NC_CACHE["nc"]

    Am = np.tril(A)
    Bm = np.tril(B)
    AT = np.ascontiguousarray(Am.T)

    TP = TL // 2
    Bblk_h = Bm.astype(np.float16).reshape(KB, P, NB)

    # Per-parity B packs: head/quad tiles in the kernel's issue order.
    def pack_b(par):
        nquad = sum(1 for J0 in range(NSLOT)
                    for k in _band_tiles(J0) if k[0] == "quad")
        npair = sum(1 for J0 in range(NSLOT)
                    for k in _band_tiles(J0) if k[0] == "pair")
        Bq = np.zeros((nquad, P, 4 * JT), np.float16)
        Bp = np.zeros((npair, P, 2 * JT), np.float16)
        Bhe = np.zeros((4, P, 256 + 3 * JT), np.float16)
        Bho = np.zeros((4, P, 256 + JT), np.float16)
        iq = ip = ihe = iho = 0
        for J0 in ORDER:
            c0 = J0 * JT
            for kind, t0, n_t, elems in _band_tiles(J0):
                if kind == "head":
                    dst = Bhe[ihe] if n_t > 2 else Bho[iho]
                    off = 0
                    for i in range(n_t):
                        w = 256 if i == 0 else JT
                        kb = 2 * (t0 + i) + par
                        dst[:, off:off + w] = Bblk_h[kb][:, c0:c0 + w]
                        off += w
                    if n_t > 2:
                        ihe += 1
                    else:
                        iho += 1
                else:
                    dst = Bq[iq] if kind == "quad" else Bp[ip]
                    for i in range(n_t):
                        kb = 2 * (t0 + i) + par
                        dst[:, i * JT:(i + 1) * JT] = \
                            Bblk_h[kb][:, c0:c0 + JT]
                    if kind == "quad":
                        iq += 1
                    else:
                        ip += 1
        return {"Bq": Bq, "Bp": Bp, "Bhe": Bhe, "Bho": Bho}

    b_par = [pack_b(0), pack_b(1)]

    in_maps = []
    for c in range(NCORES):
        par = 0 if c < 4 else 1
        cp = c % 4
        cols = np.concatenate(
            [np.arange((4 * r + cp) * P, (4 * r + cp + 1) * P)
             for r in range(NSLOT)])
        ATch = AT[:, cols].astype(np.float16)
        # parity split, then pack t-pairs onto a per-partition axis:
        # ATh[tp, p, i, m]
        m = {
            "ATh": np.ascontiguousarray(
                ATch.reshape(KB, P, NSLOT * P)[par::2]
                .reshape(TP, 2, P, NSLOT * P).transpose(0, 2, 1, 3)),
        }
        m.update(b_par[par])
        in_maps.append(m)

    cm = PROFILE_CM() if PROFILE_CM is not None else contextlib.nullcontext()
    with cm:
        res = run_bass_kernel_spmd(nc, in_maps, core_ids=list(range(NCORES)))
    LAST_RESULTS = res

    C = np.zeros((NB, NB), dtype=np.float32)
    for cp in range(4):
        even = res.results[cp]["C"]
        odd = res.results[cp + 4]["C"]
        for r in range(NSLOT):
            i = 4 * r + cp
            ncols = (r + 1) * JT
            C[i * P:(i + 1) * P, :ncols] = (
                even[r * P:(r + 1) * P, :ncols].astype(np.float32)
                + odd[r * P:(r + 1) * P, :ncols].astype(np.float32))
    return np.tril(C)


# revision 37
# speedup vs baseline: 1.0213x; 1.0213x over previous
"""Trainium2 Bass kernel for C = tril(tril(A) @ tril(B)), N=4096, fp32.

Sharding: row-parallel x 2-way k-split over 8 cores. Cores 0-3 handle
even k-blocks, cores 4-7 odd k-blocks (host sums the two partial C's).
Parity is pure data: global k-block = 2t + parity for local index t,
so one SPMD program serves both groups, fed parity-packed inputs.

Each core has 8 slots; slot r of core group member c' owns block-row
4r + c' (ibar_r = 4r+3, so slot r sweeps bands J0 = 0..r, local t in
[2*J0, 2r+1]). Smaller rows in a slot band harmlessly compute exact
zeros because A/B are pre-masked (tril) on the host.

Precision: pure fp16 inputs, fp32 PSUM accumulate, fp16 partial-C
output (host upcasts and sums the two parities). Measured rel err
~5e-4 vs the 2e-2 gate.

DMA plan (all HWDGE; only the sync+scalar rings exist on TRN2, each
sustaining ~150-165 GB/s, and both are ~100% busy for the whole body):
 - B is packed on the host into per-band tiles with big per-partition
   lines for ring efficiency: a trimmed "head" tile per band (the
   first 2 or 4 local t's, with the tril-trimmed 256-wide first step),
   pair tiles for band 0's catch-up window, and [128 x 4 x 512]
   t-quad tiles (4KB lines) for the rest.
 - The startup window is supply-bound: both rings must jointly
   deliver A's head and band 0/1's B in consumption order, so the
   loads are explicitly hand-scheduled per ring (sync_plan /
   scalar_plan) with A tiles split per t-half across the rings. The
   PE warmup burst is sized to end right when the first tiles land.
 - C partials staged fp32->fp16 by vector (DVE cast), DMA'd on the
   alternating rings; loads are issued before all outputs on each
   ring so outputs never block a load.
Bands run in ORDER ending with band 7 (single output, cast split in
two with the DMA split across both rings) so the post-matmul tail is
minimal; REN remaps each band's PSUM tags onto banks guaranteed idle
at that point.
"""
import contextlib
import numpy as np

import concourse.mybir as mybir
import concourse.tile as tile
from concourse import bacc
from concourse.bass_utils import run_bass_kernel_spmd

NB = 4096          # matrix size
P = 128            # partition / block size
KB = NB // P       # 32 global k-blocks
TL = KB // 2       # 16 local k-indices per parity
NCORES = 8
NSLOT = 8          # row-block slots per core (half-rows)
JT = 512           # band width / matmul free-dim tile

NWARM = 6          # PE clock-ramp warmup matmuls (512 free dim each)

# Band order: big bands early, tiny band 7 last (1 output tile) so the
# end-of-kernel cast+DMA tail is minimal; small bands interleaved so
# output DMAs spread. REN keeps adjacent bands' PSUM banks disjoint
# or long-drained.
ORDER = (0, 1, 2, 5, 3, 6, 4, 7)
REN = {0: 0, 1: 0, 2: 0, 5: 5, 3: 0, 6: 6, 4: 0, 7: 7}

LAST_RESULTS = None  # test harness reads exec_time_ns from here
PROFILE_CM = None    # optional: test harness sets a contextmanager factory

_NC_CACHE = {}

F16 = mybir.dt.float16
F32 = mybir.dt.float32


def _band_tiles(J0):
    """B tile schedule for band J0: ('head', t0, n_t, elems) then
    ('quad', t0) entries, in consumption order. Head covers the first
    2 (odd J0) or 4 (even J0) t's with the first step trimmed to 256
    cols; quads cover 4 t's at full 512 width (4KB lines)."""
    t0 = 2 * J0
    tiles = []
    nh = 4 if J0 % 2 == 0 else 2
    nh = min(nh, TL - t0)
    tiles.append(("head", t0, nh, 256 + (nh - 1) * JT))
    t = t0 + nh
    if J0 == 0:
        # band 0's t=4..7 are consumed while the rings are still
        # catching up -- deliver them at pair granularity
        tiles.append(("pair", t, 2, 2 * JT))
        tiles.append(("pair", t + 2, 2, 2 * JT))
        t += 4
    while t < TL:
        tiles.append(("quad", t, 4, 4 * JT))
        t += 4
    return tiles


def _build():
    nc = bacc.Bacc("TRN2")
    M = NSLOT * P  # 1024 packed A columns

    # Parity-packed inputs: local k index t on the leading axis; B is
    # packed as per-band head/quad tiles in issue order (see kernel()).
    ATh_d = nc.dram_tensor("ATh", [TL // 2, P, 2, M], F16, kind="ExternalInput")
    nquad = sum(1 for J0 in range(NSLOT) for k in _band_tiles(J0) if k[0] == "quad")
    npair = sum(1 for J0 in range(NSLOT) for k in _band_tiles(J0) if k[0] == "pair")
    Bq_d = nc.dram_tensor("Bq", [nquad, P, 4 * JT], F16, kind="ExternalInput")
    Bp_d = nc.dram_tensor("Bp", [npair, P, 2 * JT], F16, kind="ExternalInput")
    Bhe_d = nc.dram_tensor("Bhe", [4, P, 256 + 3 * JT], F16, kind="ExternalInput")
    Bho_d = nc.dram_tensor("Bho", [4, P, 256 + JT], F16, kind="ExternalInput")
    C_d = nc.dram_tensor("C", [M, NB], F16, kind="ExternalOutput")

    with tile.TileContext(nc) as tc:
        with (
            tc.tile_pool(name="ares", bufs=1) as ares,
            tc.tile_pool(name="bres", bufs=1) as bres,
            tc.tile_pool(name="obuf", bufs=24) as obuf,
            tc.tile_pool(name="psum", bufs=1, space="PSUM") as psum,
        ):
            # PE warmup: a short burst of throwaway matmuls bridges
            # the idle preamble so the HAM clock governor reaches full
            # rate by the time real data lands (a cold PE runs the
            # first ~3us of matmuls at reduced clock otherwise). warm
            # reads raw (uninitialized) SBUF: no memset dependency, so
            # the burst starts the moment the PE preamble ends; the
            # result (garbage) goes to a throwaway PSUM tile.
            warm = nc.alloc_sbuf_tensor([P, JT], F16)
            ps_w = psum.tile([P, JT], F32, tag="ps0", name="ps_warm")
            for _ in range(NWARM):
                nc.tensor.matmul(ps_w[:], warm.ap()[:, :P], warm.ap()[:],
                                 start=True, stop=True)


            # Input loads: A resident t-pair tiles + B per-band
            # head/pair/quad tiles. Tiles are declared here; the DMAs
            # are emitted below via the per-ring plans.
            ah = [ares.tile([P, 2, M], F16, tag=f"ah{tp}", name=f"ah{tp}")
                  for tp in range(TL // 2)]
            bt = {}
            bload = {}
            iq = ip = ihe = iho = 0
            border = []
            for J0 in ORDER:
                for kind, t0, n_t, elems in _band_tiles(J0):
                    b = bres.tile([P, elems], F16, tag=f"b{J0}_{t0}",
                                  name=f"b{J0}_{t0}")
                    if kind == "head":
                        src = (Bhe_d[ihe] if n_t > 2 else Bho_d[iho])[:, :elems]
                        if n_t > 2:
                            ihe += 1
                        else:
                            iho += 1
                        off = 0
                        for i in range(n_t):
                            w = 256 if i == 0 else JT
                            bt[(J0, t0 + i)] = (b, off, w)
                            off += w
                    else:
                        if kind == "quad":
                            src = Bq_d[iq, :, :]
                            iq += 1
                        else:
                            src = Bp_d[ip, :, :]
                            ip += 1
                        for i in range(n_t):
                            bt[(J0, t0 + i)] = (b, i * JT, JT)
                    bload[(J0, t0)] = (b, src, kind, n_t, elems)
                    border.append((J0, t0))

            def emit_b(key, per_t):
                b, src, kind, n_t, elems = bload[key]
                jobs = []
                if per_t and kind == "head":
                    off = 0
                    for i in range(n_t):
                        w = 256 if i == 0 else JT
                        jobs.append((w * P * 2,
                                     lambda eng, b=b, src=src, o=off, w=w:
                                     eng.dma_start(b[:, o:o + w],
                                                   src[:, o:o + w])))
                        off += w
                else:
                    jobs.append((elems * P * 2,
                                 lambda eng, b=b, src=src:
                                 eng.dma_start(b[:], src)))
                return jobs

            def a_half(tp, i):
                a = ah[tp]
                return lambda eng: eng.dma_start(
                    a[:, i, tp * P:], ATh_d[tp, :, i, tp * P:])

            def a_full(tp):
                a = ah[tp]
                return lambda eng: eng.dma_start(
                    a[:, :, tp * P:], ATh_d[tp, :, :, tp * P:])

            def a0_chunk(o0, o1):
                return lambda eng: eng.dma_start(
                    ah[0][:, 0, o0:o1], ATh_d[0, :, 0, o0:o1])

            # Explicit per-ring issue plans, hand-scheduled so every
            # tile's completion (issue ~0.65us + ring transfer at
            # ~165GB/s + ~0.3us receipt) beats its first consumption in
            # the matmul stream. Early chunks are medium-sized: smaller
            # starves the issue pipeline, bigger delays the first uses.
            h = M // 2
            b0 = emit_b((0, 0), True)           # t=0..3 singles
            sync_plan = (
                [b0[0][1], a0_chunk(h, M), b0[1][1], a_half(1, 0),
                 b0[2][1], b0[3][1],
                 emit_b((0, 4), False)[0][1],
                 emit_b((0, 6), False)[0][1],
                 emit_b((0, 8), False)[0][1],
                 emit_b((0, 12), False)[0][1]]
            )
            scalar_plan = (
                [a0_chunk(0, h), a_half(0, 1), a_half(1, 1),
                 a_half(2, 0), a_half(2, 1),
                 a_half(3, 0), a_half(3, 1), a_full(4), a_full(5)]
            )
            # remaining bands in ORDER, alternating rings; late A tiles
            # slotted onto scalar after band 1's B.
            done = {(0, t) for t in (0, 4, 6, 8, 12)}
            rest = [k for k in border if k not in done]
            late_a = [a_full(tp) for tp in (6, 7)]
            flip = 0
            for key in rest:
                fn = emit_b(key, False)[0][1]
                (scalar_plan if flip else sync_plan).append(fn)
                flip ^= 1
                if late_a:
                    scalar_plan.append(late_a.pop(0))
            # the sync (SP) ring consistently starts delivering ~1.5us
            # earlier than the scalar (Act) ring, so the A chain --
            # whose a0[1]/a1 tiles gate t=1..3 -- rides sync, and the
            # B chain rides scalar.
            for fn in sync_plan:
                fn(nc.scalar)
            for fn in scalar_plan:
                fn(nc.sync)

            nout = 0
            for J0 in ORDER:
                live = range(J0, NSLOT)
                ren = REN[J0]
                ps = {r: psum.tile([P, JT], F32, tag=f"ps{r - ren}",
                                   name=f"ps{r}_{J0}")
                      for r in live}
                for t in range(2 * J0, TL):
                    first = t == 2 * J0
                    b, off, w = bt[(J0, t)]
                    rhs = b[:, off:off + w]
                    for r in live:
                        if 2 * r + 1 < t:
                            continue
                        last = t == 2 * r + 1
                        nc.tensor.matmul(ps[r][:, :w],
                                         ah[t // 2][:, t % 2,
                                                    r * P:(r + 1) * P],
                                         rhs, start=first, stop=last)
                        if last:
                            ot = obuf.tile([P, JT], F16, tag="o",
                                           name=f"o{r}_{J0}")
                            crows = C_d[r * P:(r + 1) * P,
                                        J0 * JT:(J0 + 1) * JT]
                            if J0 == ORDER[-1]:
                                # final output: two half casts (the
                                # first DMA issues while the second
                                # half casts) with the DMA split
                                # across both rings to minimize the
                                # post-matmul tail
                                hw_ = JT // 2
                                nc.vector.tensor_copy(ot[:, :hw_],
                                                      ps[r][:, :hw_])
                                nc.sync.dma_start(crows[:, :hw_],
                                                  ot[:, :hw_])
                                nc.vector.tensor_copy(ot[:, hw_:],
                                                      ps[r][:, hw_:])
                                nc.scalar.dma_start(crows[:, hw_:],
                                                    ot[:, hw_:])
                            else:
                                nc.vector.tensor_copy(ot[:], ps[r][:])
                                oeng = nc.sync if nout % 2 else nc.scalar
                                nout += 1
                                oeng.dma_start(crows, ot[:])
    nc.finalize()
    return nc


def kernel(A, B):
    global LAST_RESULTS
    A = np.asarray(A, dtype=np.float32)
    B = np.asarray(B, dtype=np.float32)

    if "nc" not in _NC_CACHE:
        _NC_CACHE["nc"] = _build()
    nc = _NC_CACHE["nc"]

    Am = np.tril(A)
    Bm = np.tril(B)
    AT = np.ascontiguousarray(Am.T)

    TP = TL // 2
    Bblk_h = Bm.astype(np.float16).reshape(KB, P, NB)

    # Per-parity B packs: head/quad tiles in the kernel's issue order.
    def pack_b(par):
        nquad = sum(1 for J0 in range(NSLOT)
                    for k in _band_tiles(J0) if k[0] == "quad")
        npair = sum(1 for J0 in range(NSLOT)
                    for k in _band_tiles(J0) if k[0] == "pair")
        Bq = np.zeros((nquad, P, 4 * JT), np.float16)
        Bp = np.zeros((npair, P, 2 * JT), np.float16)
        Bhe = np.zeros((4, P, 256 + 3 * JT), np.float16)
        Bho = np.zeros((4, P, 256 + JT), np.float16)
        iq = ip = ihe = iho = 0
        for J0 in ORDER:
            c0 = J0 * JT
            for kind, t0, n_t, elems in _band_tiles(J0):
                if kind == "head":
                    dst = Bhe[ihe] if n_t > 2 else Bho[iho]
                    off = 0
                    for i in range(n_t):
                        w = 256 if i == 0 else JT
                        kb = 2 * (t0 + i) + par
                        dst[:, off:off + w] = Bblk_h[kb][:, c0:c0 + w]
                        off += w
                    if n_t > 2:
                        ihe += 1
                    else:
                        iho += 1
                else:
                    dst = Bq[iq] if kind == "quad" else Bp[ip]
                    for i in range(n_t):
                        kb = 2 * (t0 + i) + par
                        dst[:, i * JT:(i + 1) * JT] = \
                            Bblk_h[kb][:, c0:c0 + JT]
                    if kind == "quad":
                        iq += 1
                    else:
                        ip += 1
        return {"Bq": Bq, "Bp": Bp, "Bhe": Bhe, "Bho": Bho}

    b_par = [pack_b(0), pack_b(1)]

    in_maps = []
    for c in range(NCORES):
        par = 0 if c < 4 else 1
        cp = c % 4
        cols = np.concatenate(
            [np.arange((4 * r + cp) * P, (4 * r + cp + 1) * P)
             for r in range(NSLOT)])
        ATch = AT[:, cols].astype(np.float16)
        # parity split, then pack t-pairs onto a per-partition axis:
        # ATh[tp, p, i, m]
        m = {
            "ATh": np.ascontiguousarray(
                ATch.reshape(KB, P, NSLOT * P)[par::2]
                .reshape(TP, 2, P, NSLOT * P).transpose(0, 2, 1, 3)),
        }
        m.update(b_par[par])
        in_maps.append(m)

    cm = PROFILE_CM() if PROFILE_CM is not None else contextlib.nullcontext()
    with cm:
        res = run_bass_kernel_spmd(nc, in_maps, core_ids=list(range(NCORES)))
    LAST_RESULTS = res

    C = np.zeros((NB, NB), dtype=np.float32)
    for cp in range(4):
        even = res.results[cp]["C"]
        odd = res.results[cp + 4]["C"]
        for r in range(NSLOT):
            i = 4 * r + cp
            ncols = (r + 1) * JT
            C[i * P:(i + 1) * P, :ncols] = (
                even[r * P:(r + 1) * P, :ncols].astype(np.float32)
                + odd[r * P:(r + 1) * P, :ncols].astype(np.float32))
    return np.tril(C)
